# revision 12
# baseline (speedup 1.0000x reference)
"""Causal flash attention (B=2, H=16, S=2048, D=64, fp32) on 8 TRN2 NeuronCores.

Strategy: shard batch*heads (32) across 8 cores -> 4 heads/core. Per head,
compute transposed scores S^T[k, q] = K Q^T via PE (fp16 inputs, fp32 PSUM
accumulate), exp split across two engines (ACT spline exp; DVE Schraudolph
bit-trick exp for enough off-diagonal groups that neither engine is the
bottleneck), causal mask applied post-exp (multiplicative fp16 mask on DVE
for head A's tiles, GpSimd affine_select for head B's), then PV via PE with
a ones column appended to V so the softmax denominator falls out of the same
matmul. The output leaves the device transposed ([d+1, q] per head, fp32);
the host divides by the denominator row and transposes back.

Two heads are packed into the 128 SBUF partitions (d=64 each) so QK matmuls
for a head pair run concurrently on disjoint PE row groups.

Groups (2 k-tiles x 2 heads, one PSUM bank pair) are processed in chunks of
two: QK for both groups is one burst, the two exps go to different engines
(ACT + DVE) so they run concurrently, and the previous chunk's PV matmuls
flush as one burst — long same-shape matmul runs let the PE hide LDWEIGHTS
under the previous matmul's streaming phase. Diagonal groups go first in
their chunk so the mask chain overlaps the sibling group's exp. For the last
k-tile of each q-block only the live right half (q >= 128) is computed:
QK N=128, exp on 768 columns, PV N=128.

The DVE exp: i16 = round(s * SCALE*1024*log2e + (15360 + DELTA)), bitcast
int16->fp16 = 2^(x-15)*(1+eps) with |eps| <= 4%; DELTA=-60 centers the error
in log space so mixed ACT/DVE tiles carry no relative bias. The int16
convert is round-to-nearest-even (verified on HW).

A burst of dummy warmup matmuls at t=0 (while input DMAs are in flight)
brings the PE out of the HAM 1.2GHz cold state before real work arrives.
Both heads' PV accumulators share one PSUM bank ([128,512] tile carved into
two [65,256] halves; only the first matmul into the bank uses start=True
because start clears has_written bank-wide) so two banks double-buffer
across q-blocks.
"""

import numpy as np

B, H, S, D = 2, 16, 2048, 64
BH = B * H
NCORES = 8
HPC = BH // NCORES  # heads per core
SCALE = 0.125
W = 256             # q-block width (matmul moving dim)
HW2 = W + W // 2    # per-head col span of a diagonal (shrunk) group
TK = 128            # k-tile height
NKT = S // TK       # 16 k-tiles
NQB = S // W        # 8 q-blocks
G = 2               # k-tiles per exp group; [128, 2*G*W] fp32 = 2 PSUM banks (x3 bufs + 2 PV = 8)

# Schraudolph exp constants (fp16 bitcast): i16 = s*EXP_A + EXP_B
EXP_A = SCALE * 1024.0 / float(np.log(2.0))
EXP_B = 15360.0 - 60.0
N_WARMUP = 20       # dummy PE matmuls issued at t=0 to warm the HAM clock gate
SHRINK = True      # compute only the live half of each q-block's last k-tile
SPLIT_OUT_DMA = True

_CACHE = {}


def _build_nc():
    import concourse.bass as bass  # noqa: F401
    import concourse.mybir as mybir
    import concourse.tile as tile
    from concourse import bacc

    f32 = mybir.dt.float32
    f16 = mybir.dt.float16
    i16 = mybir.dt.int16
    EXP = mybir.ActivationFunctionType.Exp

    nc = bacc.Bacc("TRN2", target_bir_lowering=False, debug=False, num_devices=NCORES)

    qt_d = nc.dram_tensor("qt", [HPC, D, S], f16, kind="ExternalInput").ap()
    kt_d = nc.dram_tensor("kt", [HPC, D, S], f16, kind="ExternalInput").ap()
    # v arrives with a ones column pre-appended on the host ([.., D+1]).
    v_d = nc.dram_tensor("v", [HPC, S, D + 1], f16, kind="ExternalInput").ap()
    m_d = nc.dram_tensor("mask", [128, W], f16, kind="ExternalInput").ap()
    o_d = nc.dram_tensor("outT", [HPC, D + 1, S], f32, kind="ExternalOutput").ap()

    with tile.TileContext(nc) as tc:
        const_pool = tc.alloc_tile_pool(name="const", bufs=1)
        kq_pool = tc.alloc_tile_pool(name="kq", bufs=1)
        vx_pool = tc.alloc_tile_pool(name="vx", bufs=1)
        p_pool = tc.alloc_tile_pool(name="p", bufs=12)
        o_pool = tc.alloc_tile_pool(name="o", bufs=4)
        ps_pool = tc.alloc_tile_pool(name="ps", bufs=3, space="PSUM")
        pv_pool = tc.alloc_tile_pool(name="pv", bufs=2, space="PSUM")

        # --- PE warmup: dummy matmuls on a zeroed tile while inputs stream
        # in. memset on DVE.
        wu = const_pool.tile([128, 65 + W], f16, tag="wu")
        nc.vector.memset(wu[:], 0.0)
        wu_ps = ps_pool.tile([128, 2 * G * W], f32, tag="sG", name="wu_ps")
        for _ in range(N_WARMUP):
            nc.tensor.matmul(
                wu_ps[0:65, 0:W], wu[:, 0:65], wu[:, 65:65 + W],
                start=True, stop=True, skip_group_check=True,
            )

        # Causal mask (host-precomputed; GpSimd is never used so the exit
        # barrier skips its ~10us DGE drain): maskA[x, y] = 1 iff y >= x.
        maskA = const_pool.tile([128, W], f16, tag="maskA")
        nc.sync.dma_start(maskA[:], m_d)

        # Input loads. kt/qt are packed 2 heads per 128 partitions. The
        # pieces the first q-blocks touch (low k-tiles, high q columns)
        # are dispatched first, in small pieces.
        ktc = {}
        qtc = {}
        vxc = {}
        for pr in range(2):
            hA, hB = 2 * pr, 2 * pr + 1
            hsl = slice(2 * pr, 2 * pr + 2)
            kchunk = kq_pool.tile([128, S], f16, tag=f"ktc{pr}", name=f"ktc{pr}")
            qchunk = kq_pool.tile([128, S], f16, tag=f"qtc{pr}", name=f"qtc{pr}")
            ktc[pr] = kchunk
            qtc[pr] = qchunk
            for dst, src_d, sl in (
                (kchunk, kt_d, slice(0, 256)),
                (qchunk, qt_d, slice(1792, S)),
                (kchunk, kt_d, slice(256, 512)),
                (qchunk, qt_d, slice(1536, 1792)),
            ):
                nc.sync.dma_start(
                    dst[:, sl], src_d[hsl, :, sl].rearrange("h d s -> (h d) s")
                )
            for h in (hA, hB):
                vchunk = vx_pool.tile([128, NKT, D + 1], f16, tag=f"vx{h}",
                                      name=f"vx{h}")
                nc.sync.dma_start(
                    vchunk[:], v_d[h].rearrange("(j p) d -> p j d", p=128)
                )
                vxc[h] = vchunk
            nc.sync.dma_start(
                kchunk[:, 512:S],
                kt_d[hsl, :, 512:S].rearrange("h d s -> (h d) s"),
            )
            for qs in (slice(1024, 1536), slice(512, 1024), slice(0, 512)):
                nc.sync.dma_start(
                    qchunk[:, qs],
                    qt_d[hsl, :, qs].rearrange("h d s -> (h d) s"),
                )

        def ktile(pr, kt):
            return ktc[pr][:, kt * TK:(kt + 1) * TK]

        def vx(h, kt):
            return vxc[h][:, kt, :]

        # Main pipeline, one head-pair at a time.
        for pr in range(2):
            hA, hB = 2 * pr, 2 * pr + 1

            # flat group list: (qb, g0), gw == G == 2 always (nkt even).
            groups = []
            for qb in reversed(range(NQB)):
                nkt = 2 * qb + 2
                for g0 in range(0, nkt, G):
                    groups.append((qb, g0))
            # chunks of two; diagonal group (if any) first within the chunk
            chunks = []
            for c0 in range(0, len(groups), 2):
                ch = list(groups[c0:c0 + 2])
                ch.sort(key=lambda g: 0 if (g[1] + G == 2 * g[0] + 2) else 1)
                chunks.append(ch)

            pv_tiles = {}   # qb -> [128, 512] psum tile (A: cols 0:W, B: W:2W)
            pending = []    # previous chunk's list of (qb, g0, p)
            offdiag_chunk_i = 0

            def get_pv(qb):
                if qb not in pv_tiles:
                    pv_tiles[qb] = pv_pool.tile([128, 2 * W], f32, tag="pv",
                                                name=f"pv{qb}")
                return pv_tiles[qb]

            def group_tiles(qb, g0, qk):
                # tile descriptors (head_idx, kt, src_col, n, qoff) in the
                # plain layout [A0 | A1 | B0 | B1]. QK always computes full
                # tiles (N=128 QK matmuls fault on HW); for the PV/mask side
                # of a diagonal group, the last k-tile's dead left half
                # (q < 128, all above-diagonal) is simply never read, so
                # only the live right half appears (n=128, qoff=128).
                nkt = 2 * qb + 2
                if SHRINK and not qk and g0 + G == nkt:
                    hf = W // 2
                    return [(0, g0, 0, W, 0),
                            (1, g0, 2 * W, W, 0),
                            (0, g0 + 1, W + hf, hf, hf),
                            (1, g0 + 1, 3 * W + hf, hf, hf)]
                return [(0, g0, 0, W, 0),
                        (1, g0, 2 * W, W, 0),
                        (0, g0 + 1, W, W, 0),
                        (1, g0 + 1, 3 * W, W, 0)]

            def flush_pending():
                nonlocal pending
                done_qbs = []
                # PV in k-tile order (the chunk may have been diag-first
                # reordered); copies only after ALL of a q-block's PV.
                for qb, g0, p in sorted(pending, key=lambda t: (-t[0], t[1])):
                    nkt = 2 * qb + 2
                    pv = get_pv(qb)
                    for hi, kt, src, n, qoff in group_tiles(qb, g0, qk=False):
                        h = hA if hi == 0 else hB
                        pvcol = hi * W + qoff
                        # start=True clears has_written bank-wide: only
                        # the first matmul into the shared bank uses it.
                        nc.tensor.matmul(
                            pv[0:D + 1, pvcol:pvcol + n],
                            vx(h, kt),
                            p[:, src:src + n],
                            start=(kt == 0 and hi == 0 and qoff == 0),
                            stop=(kt == nkt - 1),
                            skip_group_check=True,
                        )
                    if g0 + G == nkt:
                        done_qbs.append(qb)
                for qb in done_qbs:  # q-block complete: write out
                    pv = pv_tiles[qb]
                    oAB = o_pool.tile([D + 1, 2 * W], f32, tag="o")
                    nc.vector.tensor_copy(oAB[:], pv[0:D + 1, :])
                    for h, c0 in ((hA, 0), (hB, W)):
                        nc.sync.dma_start(
                            o_d[h, 0:33, qb * W:(qb + 1) * W],
                            oAB[0:33, c0:c0 + W])
                        nc.sync.dma_start(
                            o_d[h, 33:D + 1, qb * W:(qb + 1) * W],
                            oAB[33:D + 1, c0:c0 + W])
                    del pv_tiles[qb]
                pending = []

            for chunk in chunks:
                # --- QK burst for both groups
                sgs = []
                for qb, g0 in chunk:
                    sG = ps_pool.tile([128, 2 * G * W], f32, tag="sG")
                    q0 = qb * W
                    for hi, kt, src, n, qoff in group_tiles(qb, g0, qk=True):
                        rows = slice(0, 64) if hi == 0 else slice(64, 128)
                        nc.tensor.matmul(
                            sG[:, src:src + n],
                            ktile(pr, kt)[rows],
                            qtc[pr][rows, q0 + qoff:q0 + qoff + n],
                            start=True, stop=True,
                        )
                    sgs.append(sG)

                # --- exp: diag -> ACT (+ masks); the other group -> DVE in
                # diag chunks and in every other pure-offdiag chunk.
                has_diag = any(g0 + G == 2 * qb + 2 for qb, g0 in chunk)
                if not has_diag:
                    offdiag_chunk_i += 1
                dve_left = 1 if (has_diag or offdiag_chunk_i % 2 == 1) else 0
                new_pending = []
                for (qb, g0), sG in zip(chunk, sgs):
                    nkt = 2 * qb + 2
                    is_diag = (g0 + G == nkt)
                    p = p_pool.tile([128, 2 * G * W], f16, tag="p")
                    # exp always covers the full tile, even for shrunk
                    # diagonal groups whose tail is stale garbage: the
                    # full-width read keeps every PSUM region of the bank
                    # pair ordered against future tenants' matmul writes
                    # (partial reads leave untracked regions -> fatal PSUM
                    # bank collisions). The garbage tail of p is never read.
                    use_dve = (not is_diag) and dve_left > 0
                    if use_dve:
                        dve_left -= 1
                        nc.vector.tensor_scalar(
                            p[:].bitcast(i16), sG[:],
                            EXP_A, EXP_B,
                            mybir.AluOpType.mult, mybir.AluOpType.add,
                        )
                    else:
                        nc.scalar.activation(p[:], sG[:], EXP, scale=SCALE)
                    if is_diag:
                        # causal mask (keep q >= k), multiplicative on DVE
                        for hi, kt, src, n, qoff in group_tiles(qb, g0,
                                                                qk=False):
                            if kt < nkt - 2:
                                continue  # only diagonal tiles masked
                            nc.vector.tensor_mul(
                                p[:, src:src + n], p[:, src:src + n],
                                maskA[:, 0:n])
                    new_pending.append((qb, g0, p))
                flush_pending()
                pending = new_pending
            flush_pending()

        pv_pool.release()
        ps_pool.release()
        o_pool.release()
        p_pool.release()
        vx_pool.release()
        kq_pool.release()
        const_pool.release()

    nc.compile()
    return nc


def _get_nc():
    if "nc" not in _CACHE:
        _CACHE["nc"] = _build_nc()
    return _CACHE["nc"]


def _prep_inputs(q, k, v):
    qf = np.ascontiguousarray(np.asarray(q, dtype=np.float32)).reshape(BH, S, D)
    kf = np.ascontiguousarray(np.asarray(k, dtype=np.float32)).reshape(BH, S, D)
    vf = np.ascontiguousarray(np.asarray(v, dtype=np.float32)).reshape(BH, S, D)
    vx = np.empty((BH, S, D + 1), np.float16)
    vx[:, :, :D] = vf
    vx[:, :, D] = 1.0
    qt = qf.transpose(0, 2, 1).astype(np.float16)
    kt = kf.transpose(0, 2, 1).astype(np.float16)
    maskA = np.triu(np.ones((128, W), np.float16))  # [x, y] = 1 iff y >= x
    in_maps = []
    for c in range(NCORES):
        sl = slice(HPC * c, HPC * (c + 1))
        in_maps.append({
            "qt": np.ascontiguousarray(qt[sl]),
            "kt": np.ascontiguousarray(kt[sl]),
            "v": np.ascontiguousarray(vx[sl]),
            "mask": maskA,
        })
    return in_maps


def _postprocess(results):
    out = np.empty((B, H, S, D), np.float32)
    for c in range(NCORES):
        ot = results[c]["outT"]  # [HPC, D+1, S]
        o = (ot[:, :D, :] / ot[:, D:D + 1, :]).transpose(0, 2, 1)  # [HPC, S, D]
        for i in range(HPC):
            bh = HPC * c + i
            out[bh // H, bh % H] = o[i]
    return out


def run(q, k, v, trace=False):
    from concourse.bass_utils import run_bass_kernel_spmd

    nc = _get_nc()
    in_maps = _prep_inputs(q, k, v)
    res = run_bass_kernel_spmd(
        nc, in_maps, core_ids=list(range(NCORES)), trace=trace
    )
    return _postprocess(res.results), res


def kernel(q, k, v):
    out, _ = run(q, k, v, trace=False)
    return out


# revision 13
# speedup vs baseline: 1.0910x; 1.0910x over previous
"""Causal flash attention (B=2, H=16, S=2048, D=64, fp32) on 8 TRN2 NeuronCores.

Strategy: shard batch*heads (32) across 8 cores -> 4 heads/core. Per head,
compute transposed scores S^T[k, q] = K Q^T via PE (fp16 inputs, fp32 PSUM
accumulate), exp split across two engines (ACT spline exp; DVE Schraudolph
bit-trick exp for enough off-diagonal groups that neither engine is the
bottleneck), causal mask applied post-exp (multiplicative fp16 mask on DVE
for head A's tiles, GpSimd affine_select for head B's), then PV via PE with
a ones column appended to V so the softmax denominator falls out of the same
matmul. The output leaves the device transposed ([d+1, q] per head, fp32);
the host divides by the denominator row and transposes back.

Two heads are packed into the 128 SBUF partitions (d=64 each) so QK matmuls
for a head pair run concurrently on disjoint PE row groups.

Groups (2 k-tiles x 2 heads, one PSUM bank pair) are processed in chunks of
two: QK for both groups is one burst, the two exps go to different engines
(ACT + DVE) so they run concurrently, and the previous chunk's PV matmuls
flush as one burst — long same-shape matmul runs let the PE hide LDWEIGHTS
under the previous matmul's streaming phase. Diagonal groups go first in
their chunk so the mask chain overlaps the sibling group's exp. For the last
k-tile of each q-block only the live right half (q >= 128) is computed:
QK N=128, exp on 768 columns, PV N=128.

The DVE exp: i16 = round(s * SCALE*1024*log2e + (15360 + DELTA)), bitcast
int16->fp16 = 2^(x-15)*(1+eps) with |eps| <= 4%; DELTA=-60 centers the error
in log space so mixed ACT/DVE tiles carry no relative bias. The int16
convert is round-to-nearest-even (verified on HW).

A burst of dummy warmup matmuls at t=0 (while input DMAs are in flight)
brings the PE out of the HAM 1.2GHz cold state before real work arrives.
Both heads' PV accumulators share one PSUM bank ([128,512] tile carved into
two [65,256] halves; only the first matmul into the bank uses start=True
because start clears has_written bank-wide) so two banks double-buffer
across q-blocks.
"""

import numpy as np

B, H, S, D = 2, 16, 2048, 64
BH = B * H
NCORES = 8
HPC = BH // NCORES  # heads per core
SCALE = 0.125
W = 256             # q-block width (matmul moving dim)
HW2 = W + W // 2    # per-head col span of a diagonal (shrunk) group
TK = 128            # k-tile height
NKT = S // TK       # 16 k-tiles
NQB = S // W        # 8 q-blocks
G = 2               # k-tiles per exp group; [128, 2*G*W] fp32 = 2 PSUM banks (x3 bufs + 2 PV = 8)

# Schraudolph exp constants (fp16 bitcast): i16 = s*EXP_A + EXP_B
EXP_A = SCALE * 1024.0 / float(np.log(2.0))
EXP_B = 15360.0 - 60.0
N_WARMUP = 20       # dummy PE matmuls issued at t=0 to warm the HAM clock gate
SHRINK = True      # compute only the live half of each q-block's last k-tile
SPLIT_OUT_DMA = True

_CACHE = {}


def _build_nc():
    import concourse.bass as bass  # noqa: F401
    import concourse.mybir as mybir
    import concourse.tile as tile
    from concourse import bacc

    f32 = mybir.dt.float32
    f16 = mybir.dt.float16
    i16 = mybir.dt.int16
    EXP = mybir.ActivationFunctionType.Exp

    nc = bacc.Bacc("TRN2", target_bir_lowering=False, debug=False, num_devices=NCORES)

    qt_d = nc.dram_tensor("qt", [HPC, D, S], f16, kind="ExternalInput").ap()
    kt_d = nc.dram_tensor("kt", [HPC, D, S], f16, kind="ExternalInput").ap()
    # v arrives with a ones column pre-appended on the host ([.., D+1]).
    v_d = nc.dram_tensor("v", [HPC, S, D + 1], f16, kind="ExternalInput").ap()
    m_d = nc.dram_tensor("mask", [128, W], f16, kind="ExternalInput").ap()
    # block-major output: one contiguous DMA per (pair, q-block) — each
    # sync.dma_start costs ~700ns of sync-engine descriptor generation, so
    # fewer, larger output DMAs keep the kernel tail short.
    o_d = nc.dram_tensor("outT", [2, NQB, D + 1, 2 * W], f32,
                         kind="ExternalOutput").ap()

    with tile.TileContext(nc) as tc:
        const_pool = tc.alloc_tile_pool(name="const", bufs=1)
        kq_pool = tc.alloc_tile_pool(name="kq", bufs=1)
        vx_pool = tc.alloc_tile_pool(name="vx", bufs=1)
        p_pool = tc.alloc_tile_pool(name="p", bufs=12)
        o_pool = tc.alloc_tile_pool(name="o", bufs=4)
        ps_pool = tc.alloc_tile_pool(name="ps", bufs=3, space="PSUM")
        pv_pool = tc.alloc_tile_pool(name="pv", bufs=2, space="PSUM")

        # --- PE warmup: dummy matmuls on a zeroed tile while inputs stream
        # in. memset on DVE.
        wu = const_pool.tile([128, 65 + W], f16, tag="wu")
        nc.vector.memset(wu[:], 0.0)
        wu_ps = ps_pool.tile([128, 2 * G * W], f32, tag="sG", name="wu_ps")
        for _ in range(N_WARMUP):
            nc.tensor.matmul(
                wu_ps[0:65, 0:W], wu[:, 0:65], wu[:, 65:65 + W],
                start=True, stop=True, skip_group_check=True,
            )

        # Causal mask (host-precomputed; GpSimd is never used so the exit
        # barrier skips its ~10us DGE drain): maskA[x, y] = 1 iff y >= x.
        maskA = const_pool.tile([128, W], f16, tag="maskA")
        nc.sync.dma_start(maskA[:], m_d)

        # Input loads. kt/qt are packed 2 heads per 128 partitions. The
        # pieces the first q-blocks touch (low k-tiles, high q columns)
        # are dispatched first, in small pieces.
        ktc = {}
        qtc = {}
        vxc = {}
        for pr in range(2):
            hA, hB = 2 * pr, 2 * pr + 1
            hsl = slice(2 * pr, 2 * pr + 2)
            kchunk = kq_pool.tile([128, S], f16, tag=f"ktc{pr}", name=f"ktc{pr}")
            qchunk = kq_pool.tile([128, S], f16, tag=f"qtc{pr}", name=f"qtc{pr}")
            ktc[pr] = kchunk
            qtc[pr] = qchunk
            for dst, src_d, sl in (
                (kchunk, kt_d, slice(0, 256)),
                (qchunk, qt_d, slice(1792, S)),
                (kchunk, kt_d, slice(256, 512)),
                (qchunk, qt_d, slice(1536, 1792)),
            ):
                nc.sync.dma_start(
                    dst[:, sl], src_d[hsl, :, sl].rearrange("h d s -> (h d) s")
                )
            for h in (hA, hB):
                vchunk = vx_pool.tile([128, NKT, D + 1], f16, tag=f"vx{h}",
                                      name=f"vx{h}")
                nc.sync.dma_start(
                    vchunk[:], v_d[h].rearrange("(j p) d -> p j d", p=128)
                )
                vxc[h] = vchunk
            nc.sync.dma_start(
                kchunk[:, 512:S],
                kt_d[hsl, :, 512:S].rearrange("h d s -> (h d) s"),
            )
            for qs in (slice(1024, 1536), slice(512, 1024), slice(0, 512)):
                nc.sync.dma_start(
                    qchunk[:, qs],
                    qt_d[hsl, :, qs].rearrange("h d s -> (h d) s"),
                )

        def ktile(pr, kt):
            return ktc[pr][:, kt * TK:(kt + 1) * TK]

        def vx(h, kt):
            return vxc[h][:, kt, :]

        # Main pipeline, one head-pair at a time.
        for pr in range(2):
            hA, hB = 2 * pr, 2 * pr + 1

            # flat group list: (qb, g0), gw == G == 2 always (nkt even).
            groups = []
            for qb in reversed(range(NQB)):
                nkt = 2 * qb + 2
                for g0 in range(0, nkt, G):
                    groups.append((qb, g0))
            # chunks of two; diagonal group (if any) first within the chunk
            chunks = []
            for c0 in range(0, len(groups), 2):
                ch = list(groups[c0:c0 + 2])
                ch.sort(key=lambda g: 0 if (g[1] + G == 2 * g[0] + 2) else 1)
                chunks.append(ch)

            pv_tiles = {}   # qb -> [128, 512] psum tile (A: cols 0:W, B: W:2W)
            pending = []    # previous chunk's list of (qb, g0, p)
            offdiag_chunk_i = 0

            def get_pv(qb):
                if qb not in pv_tiles:
                    pv_tiles[qb] = pv_pool.tile([128, 2 * W], f32, tag="pv",
                                                name=f"pv{qb}")
                return pv_tiles[qb]

            def group_tiles(qb, g0, qk):
                # tile descriptors (head_idx, kt, src_col, n, qoff) in the
                # plain layout [A0 | A1 | B0 | B1]. QK always computes full
                # tiles (N=128 QK matmuls fault on HW); for the PV/mask side
                # of a diagonal group, the last k-tile's dead left half
                # (q < 128, all above-diagonal) is simply never read, so
                # only the live right half appears (n=128, qoff=128).
                nkt = 2 * qb + 2
                if SHRINK and not qk and g0 + G == nkt:
                    hf = W // 2
                    return [(0, g0, 0, W, 0),
                            (1, g0, 2 * W, W, 0),
                            (0, g0 + 1, W + hf, hf, hf),
                            (1, g0 + 1, 3 * W + hf, hf, hf)]
                return [(0, g0, 0, W, 0),
                        (1, g0, 2 * W, W, 0),
                        (0, g0 + 1, W, W, 0),
                        (1, g0 + 1, 3 * W, W, 0)]

            def flush_pending():
                nonlocal pending
                done_qbs = []
                # PV in k-tile order (the chunk may have been diag-first
                # reordered); copies only after ALL of a q-block's PV.
                for qb, g0, p in sorted(pending, key=lambda t: (-t[0], t[1])):
                    nkt = 2 * qb + 2
                    pv = get_pv(qb)
                    for hi, kt, src, n, qoff in group_tiles(qb, g0, qk=False):
                        h = hA if hi == 0 else hB
                        pvcol = hi * W + qoff
                        # start=True clears has_written bank-wide: only
                        # the first matmul into the shared bank uses it.
                        nc.tensor.matmul(
                            pv[0:D + 1, pvcol:pvcol + n],
                            vx(h, kt),
                            p[:, src:src + n],
                            start=(kt == 0 and hi == 0 and qoff == 0),
                            stop=(kt == nkt - 1),
                            skip_group_check=True,
                        )
                    if g0 + G == nkt:
                        done_qbs.append(qb)
                for qb in done_qbs:  # q-block complete: write out
                    pv = pv_tiles[qb]
                    oAB = o_pool.tile([D + 1, 2 * W], f32, tag="o")
                    nc.vector.tensor_copy(oAB[:], pv[0:D + 1, :])
                    nc.sync.dma_start(o_d[pr, qb], oAB[:])
                    del pv_tiles[qb]
                pending = []

            for chunk in chunks:
                # --- QK burst for both groups
                sgs = []
                for qb, g0 in chunk:
                    sG = ps_pool.tile([128, 2 * G * W], f32, tag="sG")
                    q0 = qb * W
                    for hi, kt, src, n, qoff in group_tiles(qb, g0, qk=True):
                        rows = slice(0, 64) if hi == 0 else slice(64, 128)
                        nc.tensor.matmul(
                            sG[:, src:src + n],
                            ktile(pr, kt)[rows],
                            qtc[pr][rows, q0 + qoff:q0 + qoff + n],
                            start=True, stop=True,
                        )
                    sgs.append(sG)

                # --- exp: diag -> ACT (+ masks); the other group -> DVE in
                # diag chunks and in every other pure-offdiag chunk.
                has_diag = any(g0 + G == 2 * qb + 2 for qb, g0 in chunk)
                if not has_diag:
                    offdiag_chunk_i += 1
                dve_left = 1 if (has_diag or offdiag_chunk_i % 2 == 1) else 0
                new_pending = []
                for (qb, g0), sG in zip(chunk, sgs):
                    nkt = 2 * qb + 2
                    is_diag = (g0 + G == nkt)
                    p = p_pool.tile([128, 2 * G * W], f16, tag="p")
                    # exp always covers the full tile, even for shrunk
                    # diagonal groups whose tail is stale garbage: the
                    # full-width read keeps every PSUM region of the bank
                    # pair ordered against future tenants' matmul writes
                    # (partial reads leave untracked regions -> fatal PSUM
                    # bank collisions). The garbage tail of p is never read.
                    use_dve = (not is_diag) and dve_left > 0
                    if use_dve:
                        dve_left -= 1
                        nc.vector.tensor_scalar(
                            p[:].bitcast(i16), sG[:],
                            EXP_A, EXP_B,
                            mybir.AluOpType.mult, mybir.AluOpType.add,
                        )
                    else:
                        nc.scalar.activation(p[:], sG[:], EXP, scale=SCALE)
                    if is_diag:
                        # causal mask (keep q >= k): head A tiles on DVE
                        # (multiplicative fp16 mask), head B on GpSimd.
                        for hi, kt, src, n, qoff in group_tiles(qb, g0,
                                                                qk=False):
                            if kt < nkt - 2:
                                continue  # only diagonal tiles masked
                            if hi == 0:
                                nc.vector.tensor_mul(
                                    p[:, src:src + n], p[:, src:src + n],
                                    maskA[:, 0:n])
                            else:
                                nc.gpsimd.affine_select(
                                    out=p[:, src:src + n],
                                    in_=p[:, src:src + n],
                                    compare_op=mybir.AluOpType.is_ge,
                                    fill=0.0, base=0,
                                    pattern=[[1, n]], channel_multiplier=-1,
                                )
                    new_pending.append((qb, g0, p))
                flush_pending()
                pending = new_pending
            flush_pending()

        pv_pool.release()
        ps_pool.release()
        o_pool.release()
        p_pool.release()
        vx_pool.release()
        kq_pool.release()
        const_pool.release()

    nc.compile()
    return nc


def _get_nc():
    if "nc" not in _CACHE:
        _CACHE["nc"] = _build_nc()
    return _CACHE["nc"]


def _prep_inputs(q, k, v):
    qf = np.ascontiguousarray(np.asarray(q, dtype=np.float32)).reshape(BH, S, D)
    kf = np.ascontiguousarray(np.asarray(k, dtype=np.float32)).reshape(BH, S, D)
    vf = np.ascontiguousarray(np.asarray(v, dtype=np.float32)).reshape(BH, S, D)
    vx = np.empty((BH, S, D + 1), np.float16)
    vx[:, :, :D] = vf
    vx[:, :, D] = 1.0
    qt = qf.transpose(0, 2, 1).astype(np.float16)
    kt = kf.transpose(0, 2, 1).astype(np.float16)
    maskA = np.triu(np.ones((128, W), np.float16))  # [x, y] = 1 iff y >= x
    in_maps = []
    for c in range(NCORES):
        sl = slice(HPC * c, HPC * (c + 1))
        in_maps.append({
            "qt": np.ascontiguousarray(qt[sl]),
            "kt": np.ascontiguousarray(kt[sl]),
            "v": np.ascontiguousarray(vx[sl]),
            "mask": maskA,
        })
    return in_maps


def _postprocess(results):
    out = np.empty((B, H, S, D), np.float32)
    for c in range(NCORES):
        blob = results[c]["outT"]  # [2, NQB, D+1, 2W] block-major
        ot = np.empty((HPC, D + 1, S), np.float32)
        for pr in range(2):
            for qb in range(NQB):
                ot[2 * pr, :, qb * W:(qb + 1) * W] = blob[pr, qb, :, 0:W]
                ot[2 * pr + 1, :, qb * W:(qb + 1) * W] = blob[pr, qb, :, W:]
        o = (ot[:, :D, :] / ot[:, D:D + 1, :]).transpose(0, 2, 1)
        for i in range(HPC):
            bh = HPC * c + i
            out[bh // H, bh % H] = o[i]
    return out


def run(q, k, v, trace=False):
    from concourse.bass_utils import run_bass_kernel_spmd

    nc = _get_nc()
    in_maps = _prep_inputs(q, k, v)
    res = run_bass_kernel_spmd(
        nc, in_maps, core_ids=list(range(NCORES)), trace=trace
    )
    return _postprocess(res.results), res


def kernel(q, k, v):
    out, _ = run(q, k, v, trace=False)
    return out
